# revision 2
# baseline (speedup 1.0000x reference)
"""nn_Backwarp kernel for 8 TRN2 NeuronCores (self-contained).

kernel(image, flow) -> dense_image_warp(image, flow), computed on the 8
NeuronCores via a sharded XLA program: core c handles batch c//2, row-half
c%2 (data-parallel over B x half-H; the warp is per-pixel so no cross-core
communication). Each core receives its full batch image (gather sources can
cross the row-half boundary) plus its 256-row flow slice, computes the
bilinear backward warp (4-tap gather + lerp), and the halves are
reassembled on the host.

Note: this container's Bass ucode-gather paths (dma_gather needs the mlp
Q7 library whose load instruction does not serialize here; indirect-DMA
descriptor patching is broken under the PJRT execution path), so the
gather runs through the XLA Neuron compiler instead of a hand-written
Bass kernel.
"""

import numpy as np

B, H, W, C = 4, 512, 512, 64
OUT_ROWS = 256  # rows per core

_CACHE = {}


def _build():
    import jax
    import jax.numpy as jnp
    from jax.sharding import Mesh, PartitionSpec, NamedSharding
    from jax.experimental.shard_map import shard_map

    def body(img, fl, ybase):
        # img [1, H, W, C]; fl [1, OUT_ROWS, W, 2]; ybase [1, 1]
        img = img[0]
        fl = fl[0]
        gy = (jnp.arange(OUT_ROWS, dtype=jnp.float32) + ybase[0, 0])[:, None]
        gx = jnp.arange(W, dtype=jnp.float32)[None, :]
        qy = gy - fl[..., 0]
        qx = gx - fl[..., 1]
        fy = jnp.clip(jnp.floor(qy), 0.0, H - 2)
        fx = jnp.clip(jnp.floor(qx), 0.0, W - 2)
        ay = jnp.clip(qy - fy, 0.0, 1.0)[..., None]
        ax = jnp.clip(qx - fx, 0.0, 1.0)[..., None]
        y0 = fy.astype(jnp.int32)
        x0 = fx.astype(jnp.int32)
        flat = img.reshape(H * W, C)
        itl = y0 * W + x0
        tl = jnp.take(flat, itl, axis=0)
        tr = jnp.take(flat, itl + 1, axis=0)
        bl = jnp.take(flat, itl + W, axis=0)
        br = jnp.take(flat, itl + W + 1, axis=0)
        top = tl + ax * (tr - tl)
        bot = bl + ax * (br - bl)
        return (top + ay * (bot - top))[None]

    devs = jax.devices()[:8]
    mesh = Mesh(np.asarray(devs), ("core",))
    sh = NamedSharding(mesh, PartitionSpec("core"))
    f = jax.jit(
        shard_map(
            body,
            mesh=mesh,
            in_specs=(PartitionSpec("core"),) * 3,
            out_specs=PartitionSpec("core"),
        )
    )
    return f, sh


def kernel(image, flow):
    import jax

    image = np.ascontiguousarray(np.asarray(image, dtype=np.float32))
    flow = np.ascontiguousarray(np.asarray(flow, dtype=np.float32))
    if "f" not in _CACHE:
        _CACHE["f"], _CACHE["sh"] = _build()
    f, sh = _CACHE["f"], _CACHE["sh"]

    img8 = np.stack([image[c // 2] for c in range(8)])
    fl8 = np.stack(
        [flow[c // 2, (c % 2) * OUT_ROWS:(c % 2 + 1) * OUT_ROWS] for c in range(8)]
    )
    yb8 = np.array([[(c % 2) * float(OUT_ROWS)] for c in range(8)], np.float32)
    args = [jax.device_put(a, sh) for a in (img8, fl8, yb8)]
    out = np.asarray(f(*args))
    res = np.zeros((B, H, W, C), np.float32)
    for c in range(8):
        res[c // 2, (c % 2) * OUT_ROWS:(c % 2 + 1) * OUT_ROWS] = out[c]
    return res


# revision 3
# speedup vs baseline: 1.4741x; 1.4741x over previous
"""nn_Backwarp kernel for 8 TRN2 NeuronCores (self-contained).

kernel(image, flow) -> dense_image_warp(image, flow) on the 8 NeuronCores.

Sharding: 2D mesh (batch=4) x (row-half=2). Every input element is
uploaded exactly once (image sharded over both axes); inside the sharded
program each device all-gathers its batch's other row-half from its
sibling device (device-to-device, no host round trip), then computes the
bilinear backward warp (4-tap gather + lerp) for its own 256 output
rows. The warp is per-pixel, so there is no other cross-device
communication.

Note: this container's Bass ucode-gather paths are unusable (dma_gather
needs the mlp Q7 library whose load instruction does not serialize here;
indirect-DMA descriptor patching is broken under the PJRT execution
path), so the gather runs through the XLA Neuron compiler instead of a
hand-written Bass kernel.
"""

import numpy as np

B, H, W, C = 4, 512, 512, 64
R = 256  # output rows per core

_CACHE = {}


def _build():
    import jax
    import jax.numpy as jnp
    from jax.sharding import Mesh, PartitionSpec, NamedSharding
    from jax.experimental.shard_map import shard_map

    def body(img_half, fl, ybase):
        # img_half [1, 1, R, W, C]; fl [1, 1, R, W, 2]; ybase [1, 1]
        img = jax.lax.all_gather(img_half[0, 0], "h", axis=0, tiled=True)
        fl = fl[0, 0]
        gy = (jnp.arange(R, dtype=jnp.float32) + ybase[0, 0])[:, None]
        gx = jnp.arange(W, dtype=jnp.float32)[None, :]
        qy = gy - fl[..., 0]
        qx = gx - fl[..., 1]
        fy = jnp.clip(jnp.floor(qy), 0.0, H - 2)
        fx = jnp.clip(jnp.floor(qx), 0.0, W - 2)
        ay = jnp.clip(qy - fy, 0.0, 1.0)[..., None]
        ax = jnp.clip(qx - fx, 0.0, 1.0)[..., None]
        y0 = fy.astype(jnp.int32)
        x0 = fx.astype(jnp.int32)
        flat = img.reshape(H * W, C)
        itl = y0 * W + x0
        tl = jnp.take(flat, itl, axis=0)
        tr = jnp.take(flat, itl + 1, axis=0)
        bl = jnp.take(flat, itl + W, axis=0)
        br = jnp.take(flat, itl + W + 1, axis=0)
        top = tl + ax * (tr - tl)
        bot = bl + ax * (br - bl)
        return (top + ay * (bot - top))[None, None]

    devs = jax.devices()[:8]
    mesh = Mesh(np.asarray(devs).reshape(4, 2), ("b", "h"))
    spec = PartitionSpec("b", "h")
    sh = NamedSharding(mesh, spec)
    f = jax.jit(
        shard_map(body, mesh=mesh, in_specs=(spec, spec, spec), out_specs=spec)
    )
    return f, sh


def kernel(image, flow):
    import jax

    image = np.ascontiguousarray(np.asarray(image, dtype=np.float32))
    flow = np.ascontiguousarray(np.asarray(flow, dtype=np.float32))
    if "f" not in _CACHE:
        _CACHE["f"], _CACHE["sh"] = _build()
    f, sh = _CACHE["f"], _CACHE["sh"]

    imgs = image.reshape(B, 2, R, W, C)
    fls = flow.reshape(B, 2, R, W, 2)
    ybs = np.array([[0.0, float(R)]] * B, np.float32)
    args = [jax.device_put(a, sh) for a in (imgs, fls, ybs)]
    out = np.asarray(f(*args))
    return out.reshape(B, H, W, C)
